# revision 38
# baseline (speedup 1.0000x reference)
"""GAT (2-layer, PyG-style) on 8 Trainium2 NeuronCores.

Edge-parallel strategy (per the sharding hint) — project-then-gather:
  - Nodes are split into 8 contiguous ranges (12500/core); each core owns all
    in-edges of its nodes (~412K edges).  Per-core nodes are degree-sorted into
    128-lane tiles; slots are padded to the per-tile max degree (~3% pad).
  - Launch A (device): per-node projection R1 = x @ [W1 | W1@att_src | W1@att_dst]
    -> [h(32) | a_src(2) | a_dst(2)] per node, bf16 (once per node, not per
    edge — the naive per-edge projection costs 33x the FLOPs and 3.5x the HBM).
  - Host: gathers the 72B/edge slot payload [h_src | a_src_src | a_dst_dst]
    into a dense [128, nblocks, 36] bf16 stream (pure data movement).
  - Launch B (device), per ~256-slot chunk spanning tile boundaries:
    alpha = a_src + a_dst (DVE, bf16 2x_1p), lr = leaky_relu (DVE
    scalar_tensor_tensor — exact; the ScalarE Lrelu table costs 30x the rel
    err), e = exp (ScalarE, written twice to form [e,e] pairs so the e*h
    multiply runs packed: all operands get innermost [stride 1, count 2], the
    2x_1p requirement), V = [e*h | e,e,e,e] (DVE mult; ScalarE copies e in).
    The segment-sum over slots runs on the TensorEngine: identity-weight
    matmuls whose stride-0 output AP revisits one PSUM accumulator row
    [128, 36] per tile — PSUM read-modify-write accumulates on revisit
    (36-column spacing), <=504 moving columns per matmul (ISA limit), only
    ~3 LDWEIGHTS per tile.  Accumulators for 7 tiles share one PSUM bank;
    ScalarE copies each full bank to SBUF fp32.  Finishing (normalize by
    sum(e), +b1, ELU in bf16, R2 = elu_out @ [W2|W2@att_src2|W2@att_dst2]
    via one 4-tile PE transpose + one block-diagonal matmul) is emitted one
    chunk late per bank-group so the strict-FIFO DVE queue never stalls on
    it, and it all overlaps the remaining slot streaming.
  - Host: gathers the 8B/edge layer-2 payload [h2 | a_src2_src | a_dst2_dst].
  - Launch C (device): same pattern (1 head, 2 ch, PSUM pre-zeroed by DVE
    memset so each tile needs a single matmul), + log_softmax.
  - Pad slots gather a sentinel row with a_src = -1e6 so e == 0 exactly;
    fake lanes (padding past 12500 real nodes/core) have all-pad slots and
    their rows are dropped by the host scatter.
"""

import sys

sys.path.insert(0, "/opt/trn_rl_repo")

from contextlib import ExitStack

import ml_dtypes
import numpy as np

import concourse.tile as tile
from concourse import bass, mybir
from concourse.bass_utils import run_bass_kernel_spmd
from concourse.masks import make_identity

F32 = mybir.dt.float32
BF16 = mybir.dt.bfloat16
BF = ml_dtypes.bfloat16

NC = 8
TILE = 128
NEG_SLOPE = 0.2
BIG_NEG = -1.0e6
KC = 2  # PSUM accumulators per tile in launch C (revisit spacing = KC*4 cols)
TPB = 7  # tiles per PSUM bank in launch B (7*2*36*4B = 2016B)
SC_B = 256  # max slots per elementwise superchunk, launch B
SC_C = 1024  # max slots per elementwise superchunk, launch C

AX = mybir.AxisListType
OP = mybir.AluOpType
AF = mybir.ActivationFunctionType

_ws_seq = [0]


def _split_waits(nc, limit=1):
    """The walrus build in this container rejects instructions carrying more
    than one sem wait ("Too many sync wait commands").  Hoist excess waits
    onto NOP carriers inserted just before the instruction (same engine, same
    program order, so semantics are preserved)."""
    for f in nc.m.functions:
        for blk in f.blocks:
            il = list(blk.instructions)
            out = []
            changed = False
            for inst in il:
                si = inst.sync_info
                waits = list(si.on_wait) if (si and si.on_wait) else []
                if len(waits) > limit:
                    keep = waits[-limit:]
                    for w in waits[:-limit]:
                        _ws_seq[0] += 1
                        nop = mybir.InstNoOp(name=f"WS-{_ws_seq[0]}")
                        nop.engine = inst.engine
                        nop.sync_info = mybir.SyncInfo(on_wait=[w], on_update=[])
                        out.append(nop)
                    si.on_wait = keep
                    changed = True
                out.append(inst)
            if changed:
                blk.instructions = out


# ---------------------------------------------------------------- host prep


def _plan(src, dst, n_nodes, n_cores):
    """Node ranges, degree-sorted tiles, shared D_t schedule, slot src ids."""
    per = n_nodes // n_cores
    ntiles = (per + TILE - 1) // TILE
    padn = ntiles * TILE

    deg = np.bincount(dst, minlength=n_nodes)

    order_e = np.lexsort((src != dst, dst))
    s_src = src[order_e]
    rowptr = np.zeros(n_nodes + 1, dtype=np.int64)
    np.cumsum(deg, out=rowptr[1:])

    orders = []  # per core: global node id per sorted slot lane (-1 = fake)
    Dt_all = np.zeros((n_cores, ntiles), dtype=np.int64)
    for c in range(n_cores):
        d = deg[c * per : (c + 1) * per]
        ids = np.concatenate(
            [c * per + np.arange(per), np.full(padn - per, -1, np.int64)]
        )
        dd = np.concatenate([d, np.zeros(padn - per, np.int64)])
        o = np.argsort(dd, kind="stable")
        orders.append(ids[o])
        Dt_all[c] = dd[o].reshape(ntiles, TILE).max(axis=1)
    Dt = Dt_all.max(axis=0)
    Dt = Dt + (Dt & 1)  # even D so the PE group structure has no tail
    Dt = np.maximum(Dt, 2)
    nblocks = int(Dt.sum())

    # slot src ids per core: [nblocks, TILE] int64, pad = n_nodes
    slot_src = np.full((n_cores, nblocks, TILE), n_nodes, dtype=np.int64)
    for c in range(n_cores):
        ids = orders[c]
        b0 = 0
        for t in range(ntiles):
            D = int(Dt[t])
            nid = ids[t * TILE : (t + 1) * TILE]
            real = nid >= 0
            nid_c = np.where(real, nid, 0)
            degs = np.where(real, deg[nid_c], 0)
            jj = np.arange(D)[:, None]  # [D, TILE]
            valid = jj < degs[None, :]
            eidx = rowptr[nid_c][None, :] + np.minimum(jj, np.maximum(degs - 1, 0))
            vals = s_src[np.clip(eidx, 0, len(s_src) - 1)]
            slot_src[c, b0 : b0 + D] = np.where(valid, vals, n_nodes)
            b0 += D
    return per, ntiles, padn, Dt, nblocks, slot_src, orders


def _chunks(Dt, cap):
    """Group consecutive tiles into superchunks of at most `cap` slots."""
    groups = []
    cur = []
    s = 0
    for t, D in enumerate(Dt):
        D = int(D)
        if cur and s + D > cap:
            groups.append(cur)
            cur = []
            s = 0
        cur.append(t)
        s += D
    if cur:
        groups.append(cur)
    return groups


# ------------------------------------------------------------- launch A


def _build_a(padn, ntiles, fdim, ra, repeat=None):
    """R1 = x @ [W1 | W1@att_src | W1@att_dst] per node (tiles of 128)."""
    nc = bass.Bass("TRN2")
    xa = nc.declare_dram_parameter("xa", [fdim, padn], BF16, isOutput=False)
    w1pa = nc.declare_dram_parameter("w1pa", [fdim, ra], BF16, isOutput=False)
    r1 = nc.declare_dram_parameter("r1", [TILE, ntiles, ra], BF16, isOutput=True)

    with ExitStack() as ctx:
        tc = ctx.enter_context(tile.TileContext(nc))
        const = ctx.enter_context(tc.tile_pool(name="const", bufs=1))
        ppool = ctx.enter_context(tc.tile_pool(name="pp", bufs=4, space="PSUM"))
        outp = ctx.enter_context(tc.tile_pool(name="op", bufs=1))

        w1t = const.tile([fdim, ra], BF16)
        nc.sync.dma_start(out=w1t[:], in_=w1pa[:])
        xat = const.tile([fdim, padn], BF16)
        nc.sync.dma_start(out=xat[:], in_=xa[:])

        if repeat:
            ctx.enter_context(tc.For_i(0, repeat, 1))
        r1all = outp.tile([TILE, ntiles, ra], BF16, tag="r1all")
        for g0 in range(0, ntiles, 4):
            gn = min(4, ntiles - g0)
            ps = ppool.tile([TILE, 4, ra], F32, tag="ps")
            for i in range(gn):
                t = g0 + i
                nc.tensor.matmul(
                    out=ps[:, i, :],
                    lhsT=xat[:, t * TILE : (t + 1) * TILE],
                    rhs=w1t[:],
                    start=True,
                    stop=True,
                )
            nc.scalar.activation(
                out=r1all[:, g0 : g0 + gn, :], in_=ps[:, 0:gn, :], func=AF.Copy
            )
        nc.sync.dma_start(out=r1[:], in_=r1all[:])
    return nc


# ------------------------------------------------------------- launch B


def _build_b(nblocks, ntiles, Dt, padn, d1, nh, repeat=None):
    """Layer 1 from gathered [h(32)|a_src(2)|a_dst(2)] slots; outputs
    R2 = [h2(2) | a_src2 | a_dst2] per node."""
    ch = d1 // nh  # 16
    nc = bass.Bass("TRN2")
    xe1 = nc.declare_dram_parameter("xe1", [TILE, nblocks, d1 + 4], BF16, isOutput=False)
    b1r = nc.declare_dram_parameter("b1r", [TILE, d1], BF16, isOutput=False)
    # block-diagonal [W2|W2@a2s|W2@a2d]: p-block i (32 rows) holds cols 4i:4i+4
    w2bd = nc.declare_dram_parameter("w2bd", [4 * d1, 16], BF16, isOutput=False)
    r2 = nc.declare_dram_parameter("r2", [TILE, ntiles, 4], BF16, isOutput=True)

    groups = _chunks(Dt, SC_B)
    smax = max(sum(int(Dt[t]) for t in g) for g in groups)
    rec = d1 + 4  # 36
    nbg = (ntiles + TPB - 1) // TPB  # PSUM bank-groups

    with ExitStack() as ctx:
        tc = ctx.enter_context(tile.TileContext(nc))
        const = ctx.enter_context(tc.tile_pool(name="const", bufs=1))
        xe = ctx.enter_context(tc.tile_pool(name="xe", bufs=4))
        vpool = ctx.enter_context(tc.tile_pool(name="vp", bufs=3))
        work = ctx.enter_context(tc.tile_pool(name="wk", bufs=2))
        acl = ctx.enter_context(tc.tile_pool(name="ac", bufs=2))
        ppool = ctx.enter_context(tc.tile_pool(name="pp", bufs=2, space="PSUM"))
        tpool = ctx.enter_context(tc.tile_pool(name="tp", bufs=2, space="PSUM"))
        fin = ctx.enter_context(tc.tile_pool(name="fin", bufs=1))

        b1t = const.tile([TILE, d1], BF16)
        nc.sync.dma_start(out=b1t[:], in_=b1r[:])
        w2bt = const.tile([4 * d1, 16], BF16)
        nc.sync.dma_start(out=w2bt[:], in_=w2bd[:])
        identb = const.tile([TILE, TILE], BF16)
        make_identity(nc, identb[:])

        if repeat:
            ctx.enter_context(tc.For_i(0, repeat, 1))
        r2all = fin.tile([TILE, ntiles, 4], BF16, tag="r2all")

        def _finish(t0, t1, accf):
            """Normalize + ELU + R2 for tiles [t0, t1) (one PSUM bank-group),
            overlapped with the later slot chunks."""
            n = t1 - t0
            inv = acl.tile([TILE, TPB, 4], F32, tag="inv")
            nc.vector.tensor_scalar_add(
                out=inv[:, 0:n, :], in0=accf[:, 0:n, d1 : d1 + 4], scalar1=1e-16
            )
            nc.vector.reciprocal(out=inv[:, 0:n, :], in_=inv[:, 0:n, :])
            o1 = acl.tile([TILE, TPB, d1], BF16, tag="o1")
            nc.vector.tensor_tensor(
                out=o1[:, 0:n, :].rearrange("p t (h c) -> p t h c", h=nh),
                in0=accf[:, 0:n, 0:d1].rearrange("p t (h c) -> p t h c", h=nh),
                in1=inv[:, 0:n, :].rearrange("p t (h two) -> p t h two", h=nh)
                [:, :, :, 0:1].to_broadcast([TILE, n, nh, ch]),
                op=OP.mult,
            )
            nc.vector.tensor_tensor(
                out=o1[:, 0:n, :],
                in0=o1[:, 0:n, :],
                in1=b1t[:].unsqueeze(1).to_broadcast([TILE, n, d1]),
                op=OP.add,
            )
            # elu = max(x,0) + exp(min(x,0)) - 1
            e1 = acl.tile([TILE, TPB, d1], BF16, tag="e1")
            nc.vector.tensor_scalar_min(
                out=e1[:, 0:n, :], in0=o1[:, 0:n, :], scalar1=0.0
            )
            nc.scalar.activation(out=e1[:, 0:n, :], in_=e1[:, 0:n, :], func=AF.Exp)
            nc.vector.tensor_scalar_add(
                out=e1[:, 0:n, :], in0=e1[:, 0:n, :], scalar1=-1.0
            )
            nc.vector.tensor_scalar_max(
                out=o1[:, 0:n, :], in0=o1[:, 0:n, :], scalar1=0.0
            )
            nc.vector.tensor_tensor(
                out=o1[:, 0:n, :], in0=o1[:, 0:n, :], in1=e1[:, 0:n, :], op=OP.add
            )
            # R2: transpose 4 tiles at once, then one block-diagonal matmul
            for g0 in range(0, n, 4):
                gn = min(4, n - g0)
                pt = tpool.tile([TILE, TILE], BF16, tag="pt")
                nc.tensor.transpose(
                    out=pt[0 : gn * d1, :],
                    in_=o1[:, g0 : g0 + gn, :],
                    identity=identb[:],
                )
                o1t = work.tile([TILE, TILE], BF16, tag="o1t")
                nc.scalar.activation(
                    out=o1t[0 : gn * d1, :], in_=pt[0 : gn * d1, :], func=AF.Copy
                )
                r2p = tpool.tile([TILE, 16], F32, tag="r2p")
                nc.tensor.matmul(
                    out=r2p[:, 0 : 4 * gn],
                    lhsT=o1t[0 : gn * d1, :],
                    rhs=w2bt[0 : gn * d1, 0 : 4 * gn],
                    start=True,
                    stop=True,
                )
                nc.scalar.activation(
                    out=r2all[:, t0 + g0 : t0 + g0 + gn, :],
                    in_=r2p[:, 0 : 4 * gn],
                    func=AF.Copy,
                )

        # chunked elementwise + per-tile PE segment-sum
        acc = None
        pending = []  # (t0, t1, accs_g) awaiting deferred _finish emission
        blk = 0
        for g in groups:
            S = sum(int(Dt[t]) for t in g)
            xt = xe.tile([TILE, smax, rec], BF16, tag="xt")
            h = S // 2  # two DMAs -> parallel queues
            nc.sync.dma_start(out=xt[:, 0:h, :], in_=xe1[:, blk : blk + h, :])
            nc.sync.dma_start(out=xt[:, h:S, :], in_=xe1[:, blk + h : blk + S, :])
            al = work.tile([TILE, smax, 2], BF16, tag="al")
            nc.vector.tensor_tensor(
                out=al[:, 0:S, :],
                in0=xt[:, 0:S, d1 : d1 + 2],
                in1=xt[:, 0:S, d1 + 2 : d1 + 4],
                op=OP.add,
            )
            # lr = leaky_relu(alpha) = (alpha*slope) max alpha   (DVE STT;
            # exact — the ScalarE Lrelu table costs ~30x the rel err)
            lr = work.tile([TILE, smax, 2], BF16, tag="lr")
            nc.vector.scalar_tensor_tensor(
                out=lr[:, 0:S, :],
                in0=al[:, 0:S, :],
                scalar=NEG_SLOPE,
                in1=al[:, 0:S, :],
                op0=OP.mult,
                op1=OP.max,
            )
            # E = exp(lr) written twice -> [e,e] pairs per head (ScalarE)
            E = work.tile([TILE, smax, 2, 2], BF16, tag="E")
            nc.scalar.activation(out=E[:, 0:S, :, 0], in_=lr[:, 0:S, :], func=AF.Exp)
            nc.scalar.activation(out=E[:, 0:S, :, 1], in_=lr[:, 0:S, :], func=AF.Exp)
            # V = [e*h | e,e,e,e]
            V = vpool.tile([TILE, smax, rec], BF16, tag="V")
            nc.vector.tensor_tensor(
                out=V[:, 0:S, 0:d1].rearrange(
                    "p s (h m two) -> p s h m two", h=nh, two=2
                ),
                in0=xt[:, 0:S, 0:d1].rearrange(
                    "p s (h m two) -> p s h m two", h=nh, two=2
                ),
                in1=E[:, 0:S, :, :].unsqueeze(3).to_broadcast(
                    [TILE, S, nh, ch // 2, 2]
                ),
                op=OP.mult,
            )
            nc.scalar.activation(
                out=V[:, 0:S, d1 : d1 + 4],
                in_=E[:, 0:S, :, :].rearrange("p s h two -> p s (h two)"),
                func=AF.Copy,
            )
            # deferred finishing of the previous bank-group: by now its PE
            # accumulation + ScalarE copy are long done, so the DVE fold does
            # not stall the (strict FIFO) vector queue
            while pending:
                _finish(*pending.pop(0))
            # PE: segment-sum per tile into shared-bank PSUM accumulators
            o = 0
            for t in g:
                D = int(Dt[t])
                ti = t % TPB
                if ti == 0:
                    acc = ppool.tile([TILE, TPB, rec], F32, tag="acc")
                nc.tensor.matmul(
                    out=acc[:, ti, :],
                    lhsT=identb[:],
                    rhs=V[:, o : o + 1, :],
                    start=True,
                    stop=False,
                )
                # remaining slots, <=504 moving columns per matmul
                for p0 in range(1, D, 14):
                    pn = min(14, D - p0)
                    nc.tensor.matmul(
                        out=acc[:, ti : ti + 1, :].to_broadcast([TILE, pn, rec]),
                        lhsT=identb[:],
                        rhs=V[:, o + p0 : o + p0 + pn, :],
                        start=False,
                        stop=(p0 + pn >= D),
                        skip_group_check=True,
                    )
                o += D
                if ti == TPB - 1 or t == ntiles - 1:
                    t0 = t - ti
                    accs_g = acl.tile([TILE, TPB, rec], F32, tag="accs")
                    nc.scalar.activation(
                        out=accs_g[:, 0 : ti + 1, :],
                        in_=acc[:, 0 : ti + 1, :],
                        func=AF.Copy,
                    )
                    pending.append((t0, t + 1, accs_g))
            blk += S

        while pending:
            _finish(*pending.pop(0))
        nc.sync.dma_start(out=r2[:], in_=r2all[:])
    return nc


# ------------------------------------------------------------- launch C


def _build_c(nblocks, ntiles, Dt, padn, repeat=None):
    """Layer 2 (1 head, 2 ch) from gathered [h2(2)|a_src2|a_dst2] slots,
    plus bias and log_softmax."""
    nc = bass.Bass("TRN2")
    xe2 = nc.declare_dram_parameter("xe2", [TILE, nblocks, 4], BF16, isOutput=False)
    b2r = nc.declare_dram_parameter("b2r", [TILE, 2], F32, isOutput=False)
    y = nc.declare_dram_parameter("y", [TILE, ntiles, 2], F32, isOutput=True)

    groups = _chunks(Dt, SC_C)
    smax = max(sum(int(Dt[t]) for t in g) for g in groups)
    tpb = 49  # tiles per PSUM bank: 49*2*4*4B = 1568B

    with ExitStack() as ctx:
        tc = ctx.enter_context(tile.TileContext(nc))
        const = ctx.enter_context(tc.tile_pool(name="const", bufs=1))
        xe = ctx.enter_context(tc.tile_pool(name="xe", bufs=3))
        vpool = ctx.enter_context(tc.tile_pool(name="vp", bufs=2))
        work = ctx.enter_context(tc.tile_pool(name="wk", bufs=2))
        ppool = ctx.enter_context(tc.tile_pool(name="pp", bufs=2, space="PSUM"))
        fin = ctx.enter_context(tc.tile_pool(name="fin", bufs=1))

        b2t = const.tile([TILE, 2], F32)
        nc.sync.dma_start(out=b2t[:], in_=b2r[:])
        identb = const.tile([TILE, TILE], BF16)
        make_identity(nc, identb[:])

        if repeat:
            ctx.enter_context(tc.For_i(0, repeat, 1))
        accs = fin.tile([TILE, ntiles, KC, 4], F32, tag="accs")
        acc = None
        blk = 0
        for g in groups:
            S = sum(int(Dt[t]) for t in g)
            xt = xe.tile([TILE, smax, 4], BF16, tag="xt")
            nc.sync.dma_start(out=xt[:, 0:S, :], in_=xe2[:, blk : blk + S, :])
            al = work.tile([TILE, smax], BF16, tag="al")
            nc.vector.tensor_tensor(
                out=al[:, 0:S],
                in0=xt[:, 0:S, 2],
                in1=xt[:, 0:S, 3],
                op=OP.add,
            )
            lr = work.tile([TILE, smax], BF16, tag="lr")
            nc.vector.scalar_tensor_tensor(
                out=lr[:, 0:S],
                in0=al[:, 0:S],
                scalar=NEG_SLOPE,
                in1=al[:, 0:S],
                op0=OP.mult,
                op1=OP.max,
            )
            E = work.tile([TILE, smax, 2], BF16, tag="E")
            nc.scalar.activation(out=E[:, 0:S, 0], in_=lr[:, 0:S], func=AF.Exp)
            nc.scalar.activation(out=E[:, 0:S, 1], in_=lr[:, 0:S], func=AF.Exp)
            V = vpool.tile([TILE, smax, 4], BF16, tag="V")
            nc.vector.tensor_tensor(
                out=V[:, 0:S, 0:2],
                in0=xt[:, 0:S, 0:2],
                in1=E[:, 0:S, :],
                op=OP.mult,
            )
            nc.vector.tensor_copy(out=V[:, 0:S, 2:4], in_=E[:, 0:S, :])
            o = 0
            for t in g:
                D = int(Dt[t])
                ti = t % tpb
                if ti == 0:
                    acc = ppool.tile([TILE, tpb, KC, 4], F32, tag="acc")
                    nc.vector.memset(acc[:], 0.0)
                G = D // KC
                nc.tensor.matmul(
                    out=acc[:, ti : ti + 1, :, :].to_broadcast([TILE, G, KC, 4]),
                    lhsT=identb[:],
                    rhs=V[:, o : o + D, :],
                    start=False,
                    stop=True,
                    skip_group_check=True,
                )
                o += D
                if ti == tpb - 1 or t == ntiles - 1:
                    t0 = t - ti
                    nc.scalar.activation(
                        out=accs[:, t0 : t + 1, :, :],
                        in_=acc[:, 0 : ti + 1, :, :],
                        func=AF.Copy,
                    )
            blk += S

        # ---- batched finishing ----
        accf = fin.tile([TILE, ntiles, 4], F32, tag="accf")
        nc.vector.tensor_reduce(
            out=accf[:],
            in_=accs[:].rearrange("p t k c -> p t c k"),
            axis=AX.X,
            op=OP.add,
        )
        inv = fin.tile([TILE, ntiles], F32, tag="inv")
        nc.vector.tensor_scalar_add(out=inv[:], in0=accf[:, :, 2], scalar1=1e-16)
        nc.vector.reciprocal(out=inv[:], in_=inv[:])
        z = fin.tile([TILE, ntiles, 2], F32, tag="z")
        nc.vector.tensor_tensor(
            out=z[:],
            in0=accf[:, :, 0:2],
            in1=inv[:].unsqueeze(-1).to_broadcast([TILE, ntiles, 2]),
            op=OP.mult,
        )
        nc.vector.tensor_tensor(
            out=z[:],
            in0=z[:],
            in1=b2t[:].unsqueeze(1).to_broadcast([TILE, ntiles, 2]),
            op=OP.add,
        )
        # log_softmax over the 2 columns
        m = fin.tile([TILE, ntiles], F32, tag="m")
        nc.vector.tensor_reduce(out=m[:], in_=z[:], axis=AX.X, op=OP.max)
        nc.vector.tensor_tensor(
            out=z[:],
            in0=z[:],
            in1=m[:].unsqueeze(-1).to_broadcast([TILE, ntiles, 2]),
            op=OP.subtract,
        )
        ez = fin.tile([TILE, ntiles, 2], F32, tag="ez")
        nc.scalar.activation(out=ez[:], in_=z[:], func=AF.Exp)
        ss = fin.tile([TILE, ntiles], F32, tag="ss")
        nc.vector.tensor_reduce(out=ss[:], in_=ez[:], axis=AX.X, op=OP.add)
        nc.scalar.activation(out=ss[:], in_=ss[:], func=AF.Ln)
        yt = fin.tile([TILE, ntiles, 2], F32, tag="yt")
        nc.vector.tensor_tensor(
            out=yt[:],
            in0=z[:],
            in1=ss[:].unsqueeze(-1).to_broadcast([TILE, ntiles, 2]),
            op=OP.subtract,
        )
        nc.sync.dma_start(out=y[:], in_=yt[:])
    return nc


# ------------------------------------------------------------------- driver


def _run_gat(x, edge_index, W1, att_src1, att_dst1, b1, W2, att_src2, att_dst2, b2,
             n_cores=NC, timing=None):
    n_nodes, fdim = x.shape
    nh, ch = att_src1.shape
    d1 = nh * ch
    ra = d1 + 4  # h | a_src(2) | a_dst(2)

    src = np.concatenate([np.asarray(edge_index[0]), np.arange(n_nodes)]).astype(
        np.int64
    )
    dst = np.concatenate([np.asarray(edge_index[1]), np.arange(n_nodes)]).astype(
        np.int64
    )

    per, ntiles, padn, Dt, nblocks, slot_src, orders = _plan(
        src, dst, n_nodes, n_cores
    )

    W1 = np.asarray(W1, np.float32)
    att_src1 = np.asarray(att_src1, np.float32)
    att_dst1 = np.asarray(att_dst1, np.float32)
    W2 = np.asarray(W2, np.float32)
    att_src2 = np.asarray(att_src2, np.float32)
    att_dst2 = np.asarray(att_dst2, np.float32)

    # fused weights: [W1 | W1@att_src (per head) | W1@att_dst]
    w_asrc1 = np.stack(
        [W1[:, h * ch : (h + 1) * ch] @ att_src1[h] for h in range(nh)], axis=1
    )  # [F, nh]
    w_adst1 = np.stack(
        [W1[:, h * ch : (h + 1) * ch] @ att_dst1[h] for h in range(nh)], axis=1
    )
    w1pa = np.concatenate([W1, w_asrc1, w_adst1], axis=1).astype(BF)  # [F, ra]
    w_asrc2 = W2 @ att_src2[0]
    w_adst2 = W2 @ att_dst2[0]
    w2p = np.concatenate(
        [W2, w_asrc2[:, None], w_adst2[:, None]], axis=1
    ).astype(np.float32)  # [d1, 4]
    w2bd = np.zeros((4 * d1, 16), np.float32)  # block-diag: 4 tiles per matmul
    for i in range(4):
        w2bd[i * d1 : (i + 1) * d1, 4 * i : 4 * i + 4] = w2p
    w2bd = w2bd.astype(BF)

    import time as _time

    # ---- launch A: per-node projection ----
    xbf = np.asarray(x, np.float32).astype(BF)
    in_maps_a = []
    dst_ids = []
    for c in range(n_cores):
        ids = orders[c]
        xa = np.zeros((padn, fdim), BF)
        real = ids >= 0
        xa[real] = xbf[ids[real]]
        in_maps_a.append(
            {"xa": np.ascontiguousarray(xa.T), "w1pa": w1pa}
        )
        dst_ids.append(np.where(ids >= 0, ids, n_nodes))

    nc_a = _build_a(padn, ntiles, fdim, ra)
    _split_waits(nc_a)
    t0 = _time.perf_counter()
    res_a = run_bass_kernel_spmd(nc_a, in_maps_a, list(range(n_cores)))
    t1 = _time.perf_counter()

    # ---- host gather: layer-1 slot payload ----
    r1tab = np.zeros((n_nodes + 1, ra), BF)
    r1tab[n_nodes, d1 : d1 + 2] = BIG_NEG  # pad row: e == 0
    for c in range(n_cores):
        ids = orders[c]
        real = ids >= 0
        # device output is [TILE, ntiles, ra]: node (t, lane) at [lane, t]
        r1m = res_a.results[c]["r1"].transpose(1, 0, 2).reshape(padn, ra)
        r1tab[ids[real]] = r1m[real]

    tile_of_block = np.repeat(np.arange(ntiles), Dt.astype(np.int64))
    b1r = np.broadcast_to(np.asarray(b1, np.float32), (TILE, d1)).astype(BF)
    in_maps_b = []
    for c in range(n_cores):
        pay = r1tab[slot_src[c]]  # [nblocks, TILE, ra]
        dslot = dst_ids[c].reshape(ntiles, TILE)[tile_of_block]  # [nblocks, TILE]
        pay[:, :, d1 + 2 : d1 + 4] = r1tab[dslot][:, :, d1 + 2 : d1 + 4]
        in_maps_b.append(
            {
                "xe1": np.ascontiguousarray(pay.transpose(1, 0, 2)),
                "b1r": b1r,
                "w2bd": w2bd,
            }
        )

    nc_b = _build_b(nblocks, ntiles, Dt, padn, d1, nh)
    _split_waits(nc_b)
    t2 = _time.perf_counter()
    res_b = run_bass_kernel_spmd(nc_b, in_maps_b, list(range(n_cores)))
    t3 = _time.perf_counter()

    # ---- host gather: layer-2 slot payload ----
    r2tab = np.zeros((n_nodes + 1, 4), BF)
    r2tab[n_nodes, 2] = BIG_NEG
    for c in range(n_cores):
        ids = orders[c]
        real = ids >= 0
        r2m = res_b.results[c]["r2"].transpose(1, 0, 2).reshape(padn, 4)
        r2tab[ids[real]] = r2m[real]

    b2r = np.broadcast_to(np.asarray(b2, np.float32), (TILE, 2)).copy()
    in_maps_c = []
    for c in range(n_cores):
        pay = r2tab[slot_src[c]]  # [nblocks, TILE, 4]
        dslot = dst_ids[c].reshape(ntiles, TILE)[tile_of_block]
        pay[:, :, 3] = r2tab[dslot][:, :, 3]
        in_maps_c.append(
            {"xe2": np.ascontiguousarray(pay.transpose(1, 0, 2)), "b2r": b2r}
        )

    nc_c = _build_c(nblocks, ntiles, Dt, padn)
    _split_waits(nc_c)
    t4 = _time.perf_counter()
    res_c = run_bass_kernel_spmd(nc_c, in_maps_c, list(range(n_cores)))
    t5 = _time.perf_counter()

    if timing is not None:
        timing.update(
            la_s=t1 - t0, lb_s=t3 - t2, lc_s=t5 - t4,
            in_maps_a=in_maps_a, in_maps_b=in_maps_b, in_maps_c=in_maps_c,
        )

    out = np.zeros((n_nodes, 2), np.float32)
    for c in range(n_cores):
        ym = res_c.results[c]["y"].transpose(1, 0, 2).reshape(padn, 2)
        ids = orders[c]
        real = ids >= 0
        out[ids[real]] = ym[real]
    return out


def kernel(x, edge_index, W1, att_src1, att_dst1, b1, W2, att_src2, att_dst2, b2):
    return _run_gat(
        np.asarray(x, np.float32),
        np.asarray(edge_index),
        W1,
        att_src1,
        att_dst1,
        b1,
        W2,
        att_src2,
        att_dst2,
        b2,
    )
